# revision 1
# baseline (speedup 1.0000x reference)
"""GQA attention kernel for Trainium2, 8-core head-parallel SPMD.

Problem: B=2, T=2048, EMB=2048, 32 q-heads / 8 kv-heads (GQA, n_rep=4),
RoPE on q/k, causal softmax, output projection.

Sharding: head-parallel (tensor parallel). Core c owns q-heads 4c..4c+3 and
kv-head c: Wq/Wk/Wv column shards, Wo row shard. Each core emits a partial
out^T [EMB, B*T]; host sums the 8 partials, adds bo, transposes.

All on-device compute is in feature-major ("transposed") layout:
  xT [emb, tok] -> qT/kT/vT [dim, tok] -> scores S [qtok, ktok] (PSUM fp32)
  -> segmented softmax (per-partition stats only) -> P bf16
  -> PE-transpose P -> PV -> yT [vd, tok] -> Wo -> outT [emb, tok].

Matmul streaming dtype is bf16 (1 cyc/row on PE); softmax statistics, RoPE
and the final output stay fp32.
"""

import numpy as np
import ml_dtypes
from contextlib import ExitStack

import concourse.bass as bass
import concourse.mybir as mybir
import concourse.tile as tile

F32 = mybir.dt.float32
BF16 = mybir.dt.bfloat16

EMB = 2048
B, T = 2, 2048
TOK = B * T          # 4096
HEAD = 64
QD = 256             # per-core q dims (4 heads)
KC = 16              # emb chunks of 128
NW = 8               # token windows of 512
NT = 512
NEG = -1e30

AF = mybir.ActivationFunctionType
AX = mybir.AxisListType
OP = mybir.AluOpType


def _strip_redundant_dma_waits(nc):
    """Instruction descriptors hold few wait slots (1 for DMA, ~2 for engine
    ops); Tile emits every direct dependency as a wait. A wait (S >= v) is
    droppable when it is implied by the transitive closure of another kept
    wait: (E >= t) means the instructions contributing E's first t increments
    completed, hence their own waits held; semaphores are monotonic, so those
    conditions still hold. Keep a minimal covering subset per instruction.
    """
    from collections import defaultdict
    fn = nc.m.functions[0]
    all_insts = []
    for b in fn.blocks:
        all_insts.extend(b.instructions)

    streams = defaultdict(list)   # sem id -> [(cum_after, {wait_id: val})]
    cum = defaultdict(int)
    for ins in all_insts:
        si = ins.sync_info
        if si is None:
            continue
        wd = {}
        for w in si.on_wait:
            if str(getattr(w, "wait_mode", "sem-ge")).startswith("sem-ge"):
                wd[w.id] = max(wd.get(w.id, 0), w.wait_value)
        for u in si.on_update:
            um = str(getattr(u, "update_mode", "sem-inc"))
            if not (um.startswith("sem-inc") or um.startswith("sem-add")):
                continue
            cum[u.id] += u.update_value
            if wd:
                streams[u.id].append((cum[u.id], wd))

    def closure(pairs):
        best = dict(pairs)
        frontier = list(pairs.items())
        while frontier:
            s, v = frontier.pop()
            for cumv, wdict in streams.get(s, ()):
                if cumv > v:
                    break
                for s2, v2 in wdict.items():
                    if v2 > best.get(s2, -1):
                        best[s2] = v2
                        frontier.append((s2, v2))
        return best

    warned = 0
    for ins in all_insts:
        si = ins.sync_info
        if si is None:
            continue
        is_dma = "DMA" in type(ins).__name__
        limit = 1
        waits = list(si.on_wait)
        if not is_dma and len(waits) > 1:
            # compute engines dispatch and complete in order (PE matmuls are
            # FIFO; DVE/ACT drain per op) — a wait on the instruction's own
            # engine sem is implied by program order.
            own = {u.id for u in si.on_update}
            waits = [w for w in waits if w.id not in own] or waits[:1]
        if len(waits) <= limit:
            if len(waits) != len(si.on_wait):
                si.on_wait = waits
            continue
        if any(not str(getattr(w, "wait_mode", "sem-ge")).startswith("sem-ge")
               for w in waits):
            continue
        # fixpoint: drop any wait implied by the closure of the others
        keep = list(waits)
        changed = True
        while changed and len(keep) > 1:
            changed = False
            for w in list(keep):
                others = {}
                for x in keep:
                    if x is not w:
                        others[x.id] = max(others.get(x.id, -1), x.wait_value)
                if not others:
                    break
                if closure(others).get(w.id, -1) >= w.wait_value:
                    keep.remove(w)
                    changed = True
                    break
        if is_dma and len(keep) > 1:
            outs_refs = {getattr(a, "memref", "") for a in ins.outs}
            if "out_t" in outs_refs:
                eng = [w for w in keep
                       if not w.ant_name.startswith(("DMASW", "DMAHW"))]
                if eng:
                    keep = eng
        if len(keep) != len(si.on_wait):
            si.on_wait = keep
        if len(keep) > limit and "Drain" not in type(ins).__name__:
            warned += 1
            if warned <= 6:
                print(f"WARN {type(ins).__name__} {ins.name}: {len(keep)} waits "
                      f"{[(w.ant_name, w.wait_value) for w in keep]}")
    if warned:
        print(f"WARN: {warned} instructions still over wait limit")
    return nc


def build_nc():
    nc = bass.Bass()

    x_t = nc.declare_dram_parameter("x_t", [EMB, TOK], BF16, isOutput=False)
    wq = nc.declare_dram_parameter("wq", [EMB, QD], BF16, isOutput=False)
    wk = nc.declare_dram_parameter("wk", [EMB, HEAD], BF16, isOutput=False)
    wv = nc.declare_dram_parameter("wv", [EMB, HEAD], BF16, isOutput=False)
    wo = nc.declare_dram_parameter("wo", [QD, EMB], BF16, isOutput=False)
    bqd = nc.declare_dram_parameter("bqd", [128, 2], F32, isOutput=False)
    bkd = nc.declare_dram_parameter("bkd", [HEAD, 1], F32, isOutput=False)
    bvd = nc.declare_dram_parameter("bvd", [HEAD, 1], F32, isOutput=False)
    cosd = nc.declare_dram_parameter("cosd", [128, T], F32, isOutput=False)
    sind = nc.declare_dram_parameter("sind", [128, T], F32, isOutput=False)
    maskd = nc.declare_dram_parameter("maskd", [4, 128, NT], F32, isOutput=False)
    idb_d = nc.declare_dram_parameter("idb", [128, 128], BF16, isOutput=False)
    rtd = nc.declare_dram_parameter("rtd", [128, 128], F32, isOutput=False)
    out_t = nc.declare_dram_parameter("out_t", [EMB, TOK], F32, isOutput=True)

    with tile.TileContext(nc) as tc, ExitStack() as ctx:
        const = ctx.enter_context(tc.tile_pool(name="const", bufs=1))

        # persistent sbuf tensors
        wq_sb = const.tile([128, KC * QD], BF16, tag="wq")
        wk_sb = const.tile([128, KC * HEAD], BF16, tag="wk")
        wv_sb = const.tile([128, KC * HEAD], BF16, tag="wv")
        wo_sb = const.tile([128, 2 * EMB], BF16, tag="wo")
        bq_sb = const.tile([128, 2], F32, tag="bq")
        bk_sb = const.tile([128, 1], F32, tag="bk")
        bv_sb = const.tile([128, 1], F32, tag="bv")
        cos_sb = const.tile([128, T], F32, tag="cos")
        sin_sb = const.tile([128, T], F32, tag="sin")
        mask_sb = const.tile([128, 4 * NT], F32, tag="mask")
        idb = const.tile([128, 128], BF16, tag="idb")
        rt_sb = const.tile([128, 128], F32, tag="rt")   # rotate-half matrix (lhsT)
        qt0 = const.tile([128, TOK], BF16, tag="qt0")          # heads 0,1 (RoPE'd)
        qt1 = const.tile([128, TOK], BF16, tag="qt1")          # heads 2,3
        kt = const.tile([128, TOK], BF16, tag="kt")            # rows 64-127 dup
        vsb = const.tile([128, 32 * HEAD], BF16, tag="vsb")    # tok-major V
        yt0 = const.tile([128, TOK], BF16, tag="yt0")          # y dims 0-127
        yt1 = const.tile([128, TOK], BF16, tag="yt1")
        scrA = const.tile([128, 2], F32, tag="scrA")   # ACT wait-carrier scratch
        scrD = const.tile([128, 2], F32, tag="scrD")   # DVE wait-carrier scratch

        def ga(pool, shape, dt_, tag):
            # ACT guard absorbs the slot-release wait for ACT-first tiles
            t = pool.tile(shape, dt_, tag=tag)
            nc.scalar.activation(t[0:1, 0:1], bq_sb[0:1, 0:1], AF.Copy)
            return t

        def gd(pool, shape, dt_, tag):
            # DVE guard for DVE-first tiles
            t = pool.tile(shape, dt_, tag=tag)
            nc.vector.tensor_copy(t[0:1, 0:1], mask_sb[0:1, 0:1])
            return t

        def gtile(pool, shape, dt_, tag):
            # PE guard op takes the psum bank-release wait so the first real
            # matmul carries only its data wait (1-wait ISA budget).
            t = pool.tile(shape, dt_, tag=tag)
            if dt_ == F32:
                nc.tensor.matmul(t[0:1, 0:1], idb[0:1, 0:1], idb[0:1, 0:1],
                                 start=True, stop=True)
            else:
                nc.tensor.transpose(t[0:1, 0:1], idb[0:1, 0:1], idb[0:1, 0:1])
            return t

        # weight / table loads
        for kc in range(KC):
            nc.sync.dma_start(wq_sb[:, kc * QD:(kc + 1) * QD],
                              wq[kc * 128:(kc + 1) * 128, :])
            nc.sync.dma_start(wk_sb[:, kc * HEAD:(kc + 1) * HEAD],
                              wk[kc * 128:(kc + 1) * 128, :])
            nc.sync.dma_start(wv_sb[:, kc * HEAD:(kc + 1) * HEAD],
                              wv[kc * 128:(kc + 1) * 128, :])
        for c in range(2):
            nc.sync.dma_start(wo_sb[:, c * EMB:(c + 1) * EMB],
                              wo[c * 128:(c + 1) * 128, :])
        nc.sync.dma_start(bq_sb[:], bqd[:])
        nc.sync.dma_start(bk_sb[0:HEAD, :], bkd[:])
        nc.sync.dma_start(bv_sb[0:HEAD, :], bvd[:])
        nc.sync.dma_start(cos_sb[:], cosd[:])
        nc.sync.dma_start(sin_sb[:], sind[:])
        for j in range(4):
            nc.sync.dma_start(mask_sb[:, j * NT:(j + 1) * NT], maskd[j])
        nc.sync.dma_start(idb[:], idb_d[:])
        nc.sync.dma_start(rt_sb[:], rtd[:])

        # warm-up ops: absorb the one-time const-DMA waits on each engine so
        # steady-state instructions carry at most one wait.
        with tc.tile_pool(name="xt", bufs=8) as xpool, \
             tc.tile_pool(name="qw", bufs=2) as qwpool, \
             tc.tile_pool(name="rope", bufs=2) as rpool, \
             tc.tile_pool(name="vtmp", bufs=2) as vtpool, \
             tc.tile_pool(name="warm", bufs=1) as wpool, \
             tc.tile_pool(name="warmps", bufs=1, space="PSUM") as wps, \
             tc.tile_pool(name="pj", bufs=1, space="PSUM") as pj_ps, \
             tc.tile_pool(name="vt", bufs=1, space="PSUM") as vt_ps:
            ws_sb = wpool.tile([128, 8], F32, tag="wsb")
            wd_sb = wpool.tile([128, 8], F32, tag="wdb")
            nc.scalar.activation(ws_sb[:, 0:1], bq_sb[:, 0:1], AF.Copy)
            nc.scalar.activation(ws_sb[:, 1:2], bk_sb[:], AF.Copy)
            nc.scalar.activation(ws_sb[:, 2:3], bv_sb[:], AF.Copy)
            nc.vector.tensor_copy(wd_sb[:, 3:4], cos_sb[:, 0:1])
            nc.vector.tensor_copy(wd_sb[:, 4:5], sin_sb[:, 0:1])
            nc.vector.tensor_copy(wd_sb[:, 5:6], mask_sb[:, 0:1])
            nc.vector.tensor_copy(wd_sb[:, 6:7], mask_sb[:, NT:NT + 1])
            nc.vector.tensor_copy(wd_sb[:, 7:8], mask_sb[:, 2 * NT:2 * NT + 1])
            nc.vector.tensor_copy(wd_sb[:, 0:1], mask_sb[:, 3 * NT:3 * NT + 1])
            nc.vector.tensor_scalar_mul(ws_sb[:, 0:3], ws_sb[:, 0:3], 1.0)
            ws_ps = wps.tile([128, 8], F32, tag="wps")
            nc.tensor.matmul(ws_ps[:, 0:1], wq_sb[:, 0:128], wq_sb[:, 0:1],
                             start=True, stop=True)
            nc.tensor.matmul(ws_ps[0:HEAD, 1:2], wk_sb[:, 0:HEAD], wk_sb[:, 0:1],
                             start=False, stop=False, skip_group_check=True)
            nc.tensor.matmul(ws_ps[0:HEAD, 2:3], wv_sb[:, 0:HEAD], wv_sb[:, 0:1],
                             start=False, stop=False, skip_group_check=True)
            nc.tensor.matmul(ws_ps[:, 3:4], wo_sb[:, 0:128], wo_sb[:, 0:1],
                             start=False, stop=False, skip_group_check=True)
            nc.tensor.matmul(ws_ps[:, 4:5], idb[:], idb[:, 0:1],
                             start=False, stop=False, skip_group_check=True)
            nc.tensor.matmul(ws_ps[:, 5:6], rt_sb[:], rt_sb[:, 0:1],
                             start=False, stop=True, skip_group_check=True)

            # ------------ phase 1: QKV projections + RoPE + V transpose -----
            xt_hist = []
            for w in range(NW):
              q0p = gtile(pj_ps, [128, NT], F32, "q0")
              q1p = gtile(pj_ps, [128, NT], F32, "q1")
              kp = gtile(pj_ps, [HEAD, NT], F32, "kp")
              vp = gtile(pj_ps, [HEAD, NT], F32, "vp")
              for kc in range(KC):
                  xt = xpool.tile([128, NT], BF16, tag="xt")
                  nc.gpsimd.dma_start(xt[:], x_t[kc * 128:(kc + 1) * 128,
                                                 w * NT:(w + 1) * NT])
                  xt_hist.append(xt)
                  st, sp = kc == 0, kc == KC - 1
                  nc.tensor.matmul(q0p[:], wq_sb[:, kc * QD:kc * QD + 128],
                                   xt[:], start=st, stop=sp)
                  nc.tensor.matmul(q1p[:], wq_sb[:, kc * QD + 128:kc * QD + 256],
                                   xt[:], start=st, stop=sp)
                  nc.tensor.matmul(kp[:], wk_sb[:, kc * HEAD:(kc + 1) * HEAD],
                                   xt[:], start=st, stop=sp)
                  nc.tensor.matmul(vp[:], wv_sb[:, kc * HEAD:(kc + 1) * HEAD],
                                   xt[:], start=st, stop=sp)
              ws = slice(w * NT, (w + 1) * NT)
              # psum -> fp32 sbuf with bias add
              q0w = ga(qwpool, [128, NT], F32, "q0w")
              q1w = ga(qwpool, [128, NT], F32, "q1w")
              kw = ga(qwpool, [128, NT], F32, "kw")
              nc.scalar.activation(q0w[:], q0p[:], AF.Identity, bias=bq_sb[:, 0:1])
              nc.scalar.activation(q1w[:], q1p[:], AF.Identity, bias=bq_sb[:, 1:2])
              nc.scalar.activation(kw[0:HEAD, :], kp[:], AF.Identity,
                                   bias=bk_sb[0:HEAD, :])
              vtmp = ga(vtpool, [HEAD, NT], BF16, "vtmp")
              nc.scalar.activation(vtmp[:], vp[:], AF.Identity, bias=bv_sb[0:HEAD, :])

              # RoPE in fp32: rot(q) via PE matmul with the rotate-half matrix,
              # then q' = q*cos + rot(q)*sin; final add writes bf16.
              cs = slice((w % 4) * NT, (w % 4) * NT + NT)
              for src, dstt, np_ in ((q0w, qt0, 128), (q1w, qt1, 128), (kw, kt, HEAD)):
                  rotp = gtile(vt_ps, [128, NT], F32, "rot")
                  nc.tensor.matmul(rotp[0:np_, :], rt_sb[0:np_, 0:np_],
                                   src[0:np_, :], start=True, stop=True)
                  rsin = gd(rpool, [128, NT], F32, "rope")
                  nc.vector.tensor_mul(rsin[0:np_, :], rotp[0:np_, :], sin_sb[0:np_, cs])
                  nc.vector.tensor_mul(src[0:np_, :], src[0:np_, :], cos_sb[0:np_, cs])
                  nc.vector.tensor_add(dstt[0:np_, ws], src[0:np_, :], rsin[0:np_, :])
              # duplicate k rows for odd-head matmuls via PE (col-group 64)
              kdp = gtile(vt_ps, [128, NT], F32, "kdup")
              nc.tensor.matmul(kdp[64:128, :], idb[0:HEAD, 0:HEAD],
                               kt[0:HEAD, ws], start=True, stop=True,
                               tile_position=(0, 64))
              nc.vector.tensor_copy(kt[64:128, ws], kdp[64:128, :])
              # V -> token-major bf16 (PE transpose)
              for j in range(4):
                  vtr = gtile(vt_ps, [128, HEAD], BF16, "vtr")
                  nc.tensor.transpose(vtr[:], vtmp[:, j * 128:(j + 1) * 128],
                                      idb[0:HEAD, 0:HEAD])
                  nc.vector.tensor_copy(vsb[:, (w * 4 + j) * HEAD:(w * 4 + j + 1) * HEAD],
                                        vtr[:])

            # fence: DVE observes the last DMA on every SW lane so phase-2
            # DVE ops reusing this SBUF don't carry 8 lane waits.
            for k, t in enumerate(xt_hist[-8:]):
                nc.vector.tensor_copy(scrD[0:1, 0:1], t[0:1, 0:1])

        # ---------------- phase 2: attention ---------------------------------
        att_ctx = ExitStack()
        ppool = att_ctx.enter_context(tc.tile_pool(name="P", bufs=5))
        ptpool = att_ctx.enter_context(tc.tile_pool(name="PTsb", bufs=3))
        ytsb_pool = att_ctx.enter_context(tc.tile_pool(name="ytsb", bufs=2))
        spool = att_ctx.enter_context(tc.tile_pool(name="stats", bufs=8))
        s_ps = att_ctx.enter_context(tc.tile_pool(name="S", bufs=2, space="PSUM"))
        pt_ps = att_ctx.enter_context(tc.tile_pool(name="PT", bufs=2, space="PSUM"))
        yt_ps = att_ctx.enter_context(tc.tile_pool(name="YT", bufs=2, space="PSUM"))

        for b_i in range(B):
            for qs in range(4):
                L = qs + 1          # 512-groups in each row of this strip
                nkj = 4 * L
                for hl in range(4):
                    qtt = (qt0, qt1)[hl // 2]
                    po = 64 * (hl % 2)
                    p_tiles = []
                    for qbl in range(4):
                        qb = qs * 4 + qbl
                        ptile = ga(ppool, [128, 2048], BF16, "P")
                        stats = gd(spool, [128, 16], F32, "st")
                        segs = [list(range(L))[i:i + 2] for i in range(0, L, 2)]
                        nseg = len(segs)
                        for si, seg in enumerate(segs):
                            sps = gtile(s_ps, [128, 1024], F32, "S")
                            for gi, g in enumerate(seg):
                                nc.tensor.matmul(
                                    sps[:, gi * NT:(gi + 1) * NT],
                                    qtt[po:po + 64,
                                        b_i * T + qb * 128:b_i * T + qb * 128 + 128],
                                    kt[po:po + 64,
                                       b_i * T + g * NT:b_i * T + (g + 1) * NT],
                                    start=True, stop=True)
                            width = len(seg) * NT
                            # ACT observes the S matmuls before DVE touches S,
                            # so exp later carries only the DVE wait.
                            nc.scalar.activation(scrA[0:1, 0:1],
                                                 sps[0:1, 0:1], AF.Copy)
                            if seg[-1] == L - 1:
                                dg = slice((len(seg) - 1) * NT, width)
                                nc.vector.tensor_add(
                                    sps[:, dg], sps[:, dg],
                                    mask_sb[:, qbl * NT:(qbl + 1) * NT])
                            nc.vector.tensor_reduce(
                                stats[:, si:si + 1], sps[:, 0:width],
                                axis=AX.X, op=OP.max, negate=True)
                            nc.scalar.activation(
                                ptile[:, seg[0] * NT:seg[0] * NT + width],
                                sps[:, 0:width], AF.Exp,
                                bias=stats[:, si:si + 1],
                                accum_out=stats[:, 4 + si:5 + si])
                        if nseg == 1:
                            nc.vector.reciprocal(stats[:, 8:9], stats[:, 4:5])
                        else:
                            nc.vector.tensor_reduce(stats[:, 12:13], stats[:, 0:nseg],
                                                    axis=AX.X, op=OP.min)
                            nc.scalar.activation(stats[:, 8:8 + nseg], stats[:, 0:nseg],
                                                 AF.Exp, scale=-1.0, bias=stats[:, 12:13])
                            nc.vector.tensor_mul(stats[:, 4:4 + nseg], stats[:, 4:4 + nseg],
                                                 stats[:, 8:8 + nseg])
                            nc.vector.tensor_reduce(stats[:, 13:14], stats[:, 4:4 + nseg],
                                                    axis=AX.X, op=OP.add)
                            nc.vector.reciprocal(stats[:, 14:15], stats[:, 13:14])
                            nc.vector.tensor_scalar_mul(stats[:, 8:8 + nseg],
                                                        stats[:, 8:8 + nseg],
                                                        stats[:, 14:15])
                        for si, seg in enumerate(segs):
                            width = len(seg) * NT
                            nc.vector.tensor_scalar_mul(
                                ptile[:, seg[0] * NT:seg[0] * NT + width],
                                ptile[:, seg[0] * NT:seg[0] * NT + width],
                                stats[:, 8 + si:9 + si])
                        p_tiles.append(ptile)
                    # transpose P and accumulate PV for the whole 512-q strip;
                    # odd heads land on psum partitions 64-127 so the copy to
                    # the yt tensor is same-partition (no DMA needed).
                    po_y = 64 * (hl % 2)
                    tp = (0, 64) if po_y else None
                    ytp = gtile(yt_ps, [128, NT], F32, "YT")
                    for kj in range(nkj):
                        ptp = gtile(pt_ps, [128, NT], BF16, "PT")
                        for qbl in range(4):
                            nc.tensor.transpose(
                                ptp[:, qbl * 128:(qbl + 1) * 128],
                                p_tiles[qbl][:, kj * 128:(kj + 1) * 128], idb[:])
                        ptsb = gd(ptpool, [128, NT], BF16, "PTsb")
                        nc.vector.tensor_copy(ptsb[:], ptp[:])
                        nc.tensor.matmul(ytp[po_y:po_y + HEAD, :],
                                         vsb[:, (b_i * 16 + kj) * HEAD:(b_i * 16 + kj + 1) * HEAD],
                                         ptsb[:], start=(kj == 0), stop=(kj == nkj - 1),
                                         tile_position=tp)
                    dst = (yt0, yt1)[hl // 2]
                    nc.scalar.activation(
                        dst[po_y:po_y + HEAD,
                            b_i * T + qs * NT:b_i * T + (qs + 1) * NT],
                        ytp[po_y:po_y + HEAD, :], AF.Copy)

        # ---------------- phase 3: output projection -------------------------
        att_ctx.close()
        opool = ctx.enter_context(tc.tile_pool(name="osb", bufs=3))
        o_ps = ctx.enter_context(tc.tile_pool(name="ops", bufs=2, space="PSUM"))
        osb_hist = []
        for w in range(NW):
            ws = slice(w * NT, (w + 1) * NT)
            for m in range(KC):
                ops = gtile(o_ps, [128, NT], F32, "o")
                nc.tensor.matmul(ops[:], wo_sb[:, m * 128:(m + 1) * 128],
                                 yt0[:, ws], start=True, stop=False)
                nc.tensor.matmul(ops[:], wo_sb[:, EMB + m * 128:EMB + (m + 1) * 128],
                                 yt1[:, ws], start=False, stop=True)
                osb = opool.tile([128, NT], F32, tag="osb")
                # guard op on the same engine takes the slot-free (DMA WAR)
                # wait; the copy then carries only the PE data wait.
                if (m + w) % 2 == 0:
                    nc.scalar.activation(osb[0:1, 0:1], bq_sb[0:1, 0:1], AF.Copy)
                    nc.scalar.activation(osb[:], ops[:], AF.Copy)
                else:
                    nc.vector.tensor_copy(osb[0:1, 0:1], mask_sb[0:1, 0:1])
                    nc.vector.tensor_copy(osb[:], ops[:])
                nc.sync.dma_start(out_t[m * 128:(m + 1) * 128, ws], osb[:])
                osb_hist.append(osb)

        # end-of-kernel collectors: absorb each DMA lane's final wait on ACT
        # so the terminal drain's lane waits are implied.
        for t in osb_hist[-8:]:
            nc.scalar.activation(t[0:1, 0:1], bq_sb[0:1, 0:1], AF.Copy)

    return _strip_redundant_dma_waits(nc)


def make_in_maps(x, Wq, bq, Wk, bk, Wv, bv, Wo, bo):
    """Host-side shard + precompute. Returns list of 8 per-core input dicts."""
    bf = ml_dtypes.bfloat16
    x = np.asarray(x, np.float32)
    xT = np.ascontiguousarray(x.reshape(TOK, EMB).T).astype(bf)   # [EMB, TOK]

    inv_freq = 1.0 / (10000.0 ** (np.arange(0, HEAD, 2, dtype=np.float32) / HEAD))
    freqs = np.arange(T, dtype=np.float32)[:, None] * inv_freq[None, :]  # [T,32]
    cos_t = np.cos(freqs).astype(np.float32)                   # [T, 32]
    sin_t = np.sin(freqs).astype(np.float32)
    d = np.arange(128)
    cos2 = cos_t[:, (d % 64) % 32].T.copy()                    # [128, T]
    sinA = sin_t[:, (d % 64) % 32].T.copy()                    # [128, T]
    R64 = np.zeros((64, 64), np.float32)
    for dd in range(32):
        R64[dd, dd + 32] = -1.0
        R64[dd + 32, dd] = 1.0
    R128 = np.zeros((128, 128), np.float32)
    R128[:64, :64] = R64
    R128[64:, 64:] = R64
    rtd = np.ascontiguousarray(R128.T)

    f = np.arange(NT)
    p = np.arange(128)
    masks = np.stack([
        np.where(f[None, :] <= p[:, None] + j * 128, 0.0, NEG).astype(np.float32)
        for j in range(4)])                                    # [4,128,512]

    idb = np.eye(128).astype(bf)

    Wq = np.asarray(Wq, np.float32); Wk = np.asarray(Wk, np.float32)
    Wv = np.asarray(Wv, np.float32); Wo = np.asarray(Wo, np.float32)
    bq = np.asarray(bq, np.float32); bk = np.asarray(bk, np.float32)
    bv = np.asarray(bv, np.float32)

    scale = np.float32(1.0 / np.sqrt(HEAD))  # fold attention scale into Wq/bq
    in_maps = []
    for c in range(8):
        qs_, ks_ = slice(c * QD, (c + 1) * QD), slice(c * HEAD, (c + 1) * HEAD)
        in_maps.append({
            "x_t": xT,
            "wq": np.ascontiguousarray(Wq[:, qs_] * scale).astype(bf),
            "wk": np.ascontiguousarray(Wk[:, ks_]).astype(bf),
            "wv": np.ascontiguousarray(Wv[:, ks_]).astype(bf),
            "wo": np.ascontiguousarray(Wo[qs_, :]).astype(bf),
            "bqd": np.ascontiguousarray(bq[qs_].reshape(2, 128).T * scale),
            "bkd": bk[ks_].reshape(HEAD, 1).copy(),
            "bvd": bv[ks_].reshape(HEAD, 1).copy(),
            "cosd": cos2, "sind": sinA, "maskd": masks,
            "idb": idb, "rtd": rtd,
        })
    return in_maps


def postprocess(results, bo):
    acc = np.zeros((EMB, TOK), np.float32)
    for r in results:
        acc += r["out_t"]
    out = acc.T + np.asarray(bo, np.float32)[None, :]
    return out.reshape(B, T, EMB).astype(np.float32)


def kernel(**inputs) -> np.ndarray:
    from concourse.bass_utils import run_bass_kernel_spmd
    nc = build_nc()
    in_maps = make_in_maps(
        inputs["x"], inputs["Wq"], inputs["bq"], inputs["Wk"], inputs["bk"],
        inputs["Wv"], inputs["bv"], inputs["Wo"], inputs["bo"])
    res = run_bass_kernel_spmd(nc, in_maps, list(range(8)))
    return postprocess(res.results, inputs["bo"])

